# revision 1
# baseline (speedup 1.0000x reference)
"""AdaptiveDecayMemory kernel for 8 Trainium2 NeuronCores.

Math (per batch b):
    q = x Wq^T ; k = x Wk^T ; v = x Wv^T                       [T, D]
    scores[i,j] = (q[i].k[j]) / sqrt(D)
    decay[j] = sigmoid(x[j].Wd + bd); ld[j] = log(decay[j]+1e-8)
    w[i,j] = exp(ld[j] * max(j-i-1, 0)) * (j > i)
    out = ((scores*w) v) Wo^T * out_scale

Sharding: data-parallel over B (4 batches) x 2-way split of query rows.
Core c handles batch b = c//2 and query-row blocks (256 rows each)
m in {0,2,5,7} (c even) or {1,3,4,6} (c odd).  This interleaving makes the
upper-triangular (j > i) attention work identical across cores, so all 8
cores run one SPMD graph: position k always scans key blocks r in
[J[k], 16) with J = [0,4,8,12], which covers j > i for both cores' block
at that position; the remainder is masked to zero on device.

Layouts on device (per core):
    kT[e, j]  (feature-major)  -> lhsT for scores^T tiles
    qT[e, i]  (feature-major)  -> rhs for scores^T tiles
    v[j, e]   (token-major)    -> lhsT for retrieved^T accumulation
    ST[j, i] tiles [128, 256] in PSUM; decay weights applied with j on
    partitions (per-partition ld scale on the scalar engine).
Projections run in fp32r (full-rate, ~1e-4 matmul error), attention
matmuls in bf16 (inputs rounded once), everything accumulated in fp32.
"""

import numpy as np

B, T, D = 4, 2048, 1024
P = 128
NCORES = 8
NPOS = 4              # 256-row query blocks per core
JSTART = [0, 4, 8, 12]  # first key block (of 128) scanned at position k
MLIST = {0: [0, 2, 5, 7], 1: [1, 3, 4, 6]}  # global 256-row block ids
NMASK = 4             # tiles per position that need the j>i mask

_cache = {}


def _build_nc():
    import concourse.mybir as mybir
    from concourse import bacc
    import concourse.tile as tile

    f32 = mybir.dt.float32
    f32r = mybir.dt.float32r
    bf16 = mybir.dt.bfloat16
    Alu = mybir.AluOpType
    ACT = mybir.ActivationFunctionType

    nc = bacc.Bacc("TRN2", target_bir_lowering=False, debug=False,
                   num_devices=NCORES)

    # All big tensors arrive pre-shuffled on the host into DMA-native
    # [stage, 128, KD, cols] layouts: each stage slice is contiguous per
    # partition, so DMA runs near line rate.
    xT_d = nc.dram_tensor("xT", [4, P, D // P, 512], f32r,
                          kind="ExternalInput")
    xq_d = nc.dram_tensor("xq", [2, P, D // P, 512], f32r,
                          kind="ExternalInput")
    wq_d = nc.dram_tensor("Wqt", [2, P, D // P, 512], f32r, kind="ExternalInput")
    wk_d = nc.dram_tensor("Wkt", [2, P, D // P, 512], f32r, kind="ExternalInput")
    wv_d = nc.dram_tensor("Wvt", [2, P, D // P, 512], f32r, kind="ExternalInput")
    wo_d = nc.dram_tensor("Wot", [2, P, D // P, 512], f32r, kind="ExternalInput")
    wd_d = nc.dram_tensor("Wdt", [P, D // P, 2], f32r, kind="ExternalInput")
    negI_d = nc.dram_tensor("negI", [P, NPOS * 256], f32, kind="ExternalInput")
    bd_d = nc.dram_tensor("bd128", [P, 1], f32, kind="ExternalInput")
    os_d = nc.dram_tensor("os128", [P, 1], f32, kind="ExternalInput")
    out_d = nc.dram_tensor("out", [NPOS * 256 // P, 2, P, 512], f32,
                           kind="ExternalOutput")

    w_r = {"q": wq_d.ap(), "k": wk_d.ap(), "v": wv_d.ap(), "o": wo_d.ap()}
    blo2 = nc.dram_tensor("blo2", [T], f32)

    KD = D // P   # 8 chunks of the contraction dim
    NJ = T // P   # 16 key blocks


    with tile.TileContext(nc) as tc:
        with (
            tc.tile_pool(name="resident", bufs=1) as res,
            tc.tile_pool(name="wpool", bufs=2) as wpool,
            tc.tile_pool(name="wkq", bufs=4) as wkq,
            tc.tile_pool(name="stage", bufs=2) as stage,
            tc.tile_pool(name="spool", bufs=16) as spool,
            tc.tile_pool(name="dwpool", bufs=2) as dwpool,
            tc.tile_pool(name="mpool", bufs=1) as mpool,
            tc.tile_pool(name="rtpool", bufs=1) as rtpool,
            tc.tile_pool(name="opool", bufs=2) as opool,
            tc.tile_pool(name="small", bufs=1) as small,
            tc.tile_pool(name="ldrow", bufs=2) as ldrow_pool,
            tc.tile_pool(name="proj_ps", bufs=2, space="PSUM") as proj_ps,
            tc.tile_pool(name="st_ps", bufs=2, space="PSUM") as st_ps,
            tc.tile_pool(name="ret_ps", bufs=2, space="PSUM") as ret_ps,
            tc.tile_pool(name="out_ps", bufs=2, space="PSUM") as out_ps,
        ):
            kT = res.tile([P, KD, T], bf16)          # 32KB/part
            vtok = res.tile([P, NJ, D], bf16)        # 32KB/part
            qT = res.tile([P, KD, NPOS * 256], bf16)  # 16KB/part
            negI = res.tile([P, NPOS * 256], f32)    # 4KB/part

            def w_half(name, half):
                t = wpool.tile([P, KD, 512], f32r, tag="w")
                nc.sync.dma_start(t[:], w_r[name][half])
                return t

            def x_stage(src_r, c):
                xs = stage.tile([P, KD, 512], f32r, tag="xs")
                nc.sync.dma_start(xs[:], src_r[c])
                return xs

            # ---- Phase 1a: kT + qT projections ----
            # Wk arrives as four quarter tiles so the first matmul only
            # waits for 1MB of weights + the first x stage.
            def wk_quarter(qi):
                t = wkq.tile([P, KD, 256], f32r, tag="wkq")
                nc.sync.dma_start(
                    t[:], w_r["k"][qi // 2][:, :, (qi % 2) * 256:
                                            (qi % 2) * 256 + 256])
                return t

            def wv_quarter(qi):
                t = wkq.tile([P, KD, 256], f32r, tag="wkq")
                nc.sync.dma_start(
                    t[:], w_r["v"][qi // 2][:, :, (qi % 2) * 256:
                                            (qi % 2) * 256 + 256])
                return t

            wk_q0 = wk_quarter(0)
            xs0 = x_stage(xT_d.ap(), 0)
            wk_q = (wk_q0, wk_quarter(1), wk_quarter(2), wk_quarter(3))
            xs1 = x_stage(xT_d.ap(), 1)
            wq_t = []

            wd_t = small.tile([P, KD, 2], f32r, tag="wd")
            nc.sync.dma_start(wd_t[:], wd_d.ap())
            bd_t = small.tile([P, 1], f32, tag="bd")
            nc.sync.dma_start(bd_t[:], bd_d.ap())
            os_t = small.tile([P, 1], f32, tag="os")
            nc.sync.dma_start(os_t[:], os_d.ap())
            jall = small.tile([P, NJ], f32, tag="jall")
            nc.gpsimd.iota(jall[:], pattern=[[P, NJ]], base=0,
                           channel_multiplier=1,
                           allow_small_or_imprecise_dtypes=True)
            ldc = small.tile([P, NJ], f32, tag="ldc")
            ldT = small.tile([P, NJ], f32, tag="ldT")
            negLd = small.tile([P, NJ], f32, tag="negld")
            eps_t = small.tile([P, 1], f32, tag="eps")
            nc.vector.memset(eps_t[:], 1e-8)
            os32_t = small.tile([P, 1], f32, tag="os32")
            nc.vector.tensor_scalar_mul(os32_t[:], os_t[:],
                                        1.0 / float(np.sqrt(D)))

            for c in range(4):
                xs = (xs0, xs1, None, None)[c] or x_stage(xT_d.ap(), c)
                # Wq halves and Wv quarters queue AFTER the later x stages
                # on the DMA ring so the k-loop streaming is never stuck
                # behind them, yet they land before the q->v transition
                if c >= 2:
                    wq_t.append(w_half("q", c - 2))
                if c == 3:
                    wv_q = tuple(wv_quarter(qi) for qi in range(4))
                for oe in range(KD):
                    ps = proj_ps.tile([P, 512], f32, tag="proj")
                    wh = wk_q[oe // 2]
                    wsl = (oe % 2) * P
                    for od in range(KD):
                        nc.tensor.matmul(
                            ps[:], wh[:, od, wsl:wsl + P], xs[:, od, :],
                            start=(od == 0), stop=(od == KD - 1))
                    nc.vector.tensor_copy(
                        kT[:, oe, c * 512:(c + 1) * 512], ps[:])
                # decay logits, row form: Wd is the 2-column stationary
                # operand so LDWEIGHTS is ~free; row 1 is junk. The row
                # goes out through DRAM and is read back in column form.
                lp = st_ps.tile([P, 512], f32, tag="st")
                for od in range(KD):
                    nc.tensor.matmul(lp[0:2, :], wd_t[:, od, :],
                                     xs[:, od, :],
                                     start=(od == 0), stop=(od == KD - 1))
                lr = ldrow_pool.tile([1, 512], f32, tag="lr")
                nc.vector.tensor_copy(lr[:], lp[0:1, :])
                nc.scalar.dma_start(blo2.ap()[c * 512:(c + 1) * 512], lr[:])

            for c in range(2):
                xs = x_stage(xq_d.ap(), c)
                for oe in range(KD):
                    ps = proj_ps.tile([P, 512], f32, tag="proj")
                    wh = wq_t[oe // 4]
                    wsl = (oe % 4) * P
                    for od in range(KD):
                        nc.tensor.matmul(
                            ps[:], wh[:, od, wsl:wsl + P], xs[:, od, :],
                            start=(od == 0), stop=(od == KD - 1))
                    # fold out_scale/sqrt(D) into q (out needs no scale)
                    nc.scalar.activation(qT[:, oe, c * 512:(c + 1) * 512],
                                         ps[:], ACT.Copy, bias=0.0,
                                         scale=os32_t[:])

            # decay logits readback (column form) + decay math; ldT is
            # only needed in phase 2 so this hides under the q/v phases
            nc.scalar.dma_start(ldc[:],
                                blo2.ap().rearrange("(o p) -> p o", p=P))
            # decay: ld = log(sigmoid(raw + bd) + 1e-8), negLd = -ld
            nc.scalar.activation(ldT[:], ldc[:], ACT.Sigmoid,
                                 bias=bd_t[:], scale=1.0)
            nc.scalar.activation(ldT[:], ldT[:], ACT.Ln, bias=eps_t[:])
            nc.vector.tensor_scalar_mul(negLd[:], ldT[:], -1.0)

            # ---- Phase 1b: v (token-major) ----
            nc.sync.dma_start(negI[:], negI_d.ap())

            for c in range(4):
                xs = x_stage(xT_d.ap(), c)
                for jsub in range(4):
                    jo = 4 * c + jsub
                    xsl = xs[:, :, jsub * P:(jsub + 1) * P]
                    for ec in range(4):
                        ps = proj_ps.tile([P, 512], f32, tag="proj")
                        for od in range(KD):
                            nc.tensor.matmul(
                                ps[:, 0:256], xsl[:, od, :],
                                wv_q[ec][:, od, :],
                                start=(od == 0), stop=(od == KD - 1))
                        nc.vector.tensor_copy(
                            vtok[:, jo, ec * 256:(ec + 1) * 256],
                            ps[:, 0:256])

            # ---- Phase 2: attention + output projection per position ----
            wo_t = (w_half("o", 0), w_half("o", 1))

            for k in range(NPOS):
                isl = slice(k * 256, (k + 1) * 256)
                rlist = list(range(JSTART[k], NJ))
                s_tiles = []
                for t_idx, r in enumerate(rlist):
                    ps = st_ps.tile([P, 256], f32, tag="st")
                    for oe in range(KD):
                        nc.tensor.matmul(
                            ps[:], kT[:, oe, r * P:(r + 1) * P],
                            qT[:, oe, isl],
                            start=(oe == 0), stop=(oe == KD - 1))
                    # decay weights: dist1 = max(j - i, 0);
                    # w = exp(ld*(dist1-1)); mask = dist1 >= 1
                    dw = dwpool.tile([P, 256], f32, tag="dw")
                    nc.vector.tensor_scalar(dw[:], negI[:, isl],
                                            jall[:, r:r + 1], 0.0,
                                            Alu.add, Alu.max)
                    if t_idx < NMASK:
                        mk = mpool.tile([P, 256], f32, tag="mk")
                        nc.vector.tensor_scalar(mk[:], dw[:], 1.0, None,
                                                Alu.is_ge)
                    nc.scalar.activation(dw[:], dw[:], ACT.Exp,
                                         bias=negLd[:, r:r + 1],
                                         scale=ldT[:, r:r + 1])
                    if t_idx < NMASK:
                        nc.vector.tensor_mul(dw[:], dw[:], mk[:])
                    s_sb = spool.tile([P, 256], bf16, tag="s")
                    nc.vector.tensor_mul(s_sb[:], ps[:], dw[:])
                    s_tiles.append(s_sb)

                rt = rtpool.tile([P, KD, 256], f32r, tag="rt")
                for od in range(KD):
                    rp = ret_ps.tile([P, 256], f32, tag="ret")
                    for t_idx, r in enumerate(rlist):
                        nc.tensor.matmul(
                            rp[:], vtok[:, r, od * P:(od + 1) * P],
                            s_tiles[t_idx][:],
                            start=(t_idx == 0), stop=(t_idx == len(rlist) - 1))
                    nc.vector.tensor_copy(rt[:, od, :], rp[:])

                for isub in range(2):
                    for ec in range(2):
                        op = out_ps.tile([P, 512], f32, tag="op")
                        for od in range(KD):
                            nc.tensor.matmul(
                                op[:], rt[:, od, isub * P:(isub + 1) * P],
                                wo_t[ec][:, od, :],
                                start=(od == 0), stop=(od == KD - 1))
                        ob = opool.tile([P, 512], f32, tag="ob")
                        nc.vector.tensor_copy(ob[:], op[:])
                        nc.sync.dma_start(
                            out_d.ap()[2 * k + isub, ec], ob[:])

    nc.compile()
    return nc


def _core_rows(h):
    return np.concatenate(
        [np.arange(256 * m, 256 * (m + 1)) for m in MLIST[h]])


def _dmalayout(arrT, ch=512):
    """[D, ncols] feature-major array -> [ncols//ch, 128, D//128, ch]."""
    d, ncols = arrT.shape
    return np.ascontiguousarray(
        arrT.reshape(d // P, P, ncols // ch, ch).transpose(2, 1, 0, 3))


def make_in_maps(x, Wq, Wk, Wv, Wo, Wd, bd, out_scale):
    f = np.float32
    x = np.asarray(x, f)
    wqt = _dmalayout(np.asarray(Wq, f).T)
    wkt = _dmalayout(np.asarray(Wk, f).T)
    wvt = _dmalayout(np.asarray(Wv, f).T)
    wot = _dmalayout(np.asarray(Wo, f).T)
    wdt = np.ascontiguousarray(
        np.concatenate([np.asarray(Wd, f).reshape(1, D).T,
                        np.zeros((D, 1), f)], axis=1)
        .reshape(D // P, P, 2).swapaxes(0, 1))
    bd128 = np.full((P, 1), np.asarray(bd, f).reshape(-1)[0], f)
    os128 = np.full((P, 1), np.asarray(out_scale, f).reshape(-1)[0], f)

    in_maps = []
    rows_h = {h: _core_rows(h) for h in (0, 1)}
    negI_h = {h: np.tile(-rows_h[h].astype(f)[None, :], (P, 1))
              for h in (0, 1)}
    for c in range(NCORES):
        b, h = c // 2, c % 2
        xb = x[b]
        in_maps.append({
            "xT": _dmalayout(xb.T),
            "xq": _dmalayout(np.ascontiguousarray(xb[rows_h[h]].T)),
            "Wqt": wqt, "Wkt": wkt, "Wvt": wvt, "Wot": wot, "Wdt": wdt,
            "negI": negI_h[h], "bd128": bd128, "os128": os128,
        })
    return in_maps, rows_h


def assemble_out(results, rows_h):
    f = np.float32
    out = np.empty((B, T, D), f)
    for c in range(NCORES):
        b, h = c // 2, c % 2
        oc = results[c]["out"]  # [8, 2, 128, 512]
        out[b][rows_h[h]] = oc.transpose(0, 2, 1, 3).reshape(NPOS * 256, D)
    return out


def kernel(x, Wq, Wk, Wv, Wo, Wd, bd, out_scale):
    from concourse.bass_utils import run_bass_kernel_spmd

    if "nc" not in _cache:
        _cache["nc"] = _build_nc()
    nc = _cache["nc"]

    in_maps, rows_h = make_in_maps(x, Wq, Wk, Wv, Wo, Wd, bd, out_scale)
    res = run_bass_kernel_spmd(nc, in_maps, list(range(NCORES)))
    return assemble_out(res.results, rows_h)



# revision 5
# speedup vs baseline: 1.1839x; 1.1839x over previous
"""AdaptiveDecayMemory kernel for 8 Trainium2 NeuronCores.

Math (per batch b):
    q = x Wq^T ; k = x Wk^T ; v = x Wv^T                       [T, D]
    scores[i,j] = (q[i].k[j]) / sqrt(D)
    decay[j] = sigmoid(x[j].Wd + bd); ld[j] = log(decay[j]+1e-8)
    w[i,j] = exp(ld[j] * max(j-i-1, 0)) * (j > i)
    out = ((scores*w) v) Wo^T * out_scale

Projection folding: scores = x (Wq^T Wk) x^T and
out = (S x)(Wv^T Wo^T) * out_scale, so with M = Wq^T Wk and
N = Wv^T Wo^T (each one 1024^3 matmul, batch-independent) the K and V
projections vanish: keys/values are x itself, shipped pre-cast to bf16
in both feature-major (scores lhsT) and token-major (retrieve lhsT)
layouts.  Per-core tensor work drops from ~240us to ~190us of stream.

Sharding: data-parallel over B (4 batches) x 2-way split of query rows.
Core c handles batch b = c//2 and query-row blocks (256 rows each)
m in {0,2,5,7} (c even) or {1,3,4,6} (c odd).  This interleaving makes the
upper-triangular (j > i) attention work identical across cores, so all 8
cores run one SPMD graph: position k always scans key blocks r in
[J[k], 16) with J = [0,4,8,12], which covers j > i for both cores' block
at that position; the remainder is masked to zero on device.

Layouts on device (per core):
    xTb[e, j]  (feature-major bf16)  -> lhsT for scores^T tiles
    xtok[j, e] (token-major bf16)    -> lhsT for retrieved^T accumulation
    M/N resident f32r [d-part, chunk, 1024]
    ST[j, i] tiles [128, 256] in PSUM; decay weights applied with j on
    partitions (per-partition ld scale on the scalar engine).
M/N/q~ run in fp32r (full-rate, ~1e-4 matmul error), attention matmuls
in bf16, everything accumulated in fp32.
"""

import numpy as np

B, T, D = 4, 2048, 1024
P = 128
NCORES = 8
NPOS = 4              # 256-row query blocks per core
JSTART = [0, 4, 8, 12]  # first key block (of 128) scanned at position k
MLIST = {0: [0, 2, 5, 7], 1: [1, 3, 4, 6]}  # global 256-row block ids
NMASK = 4             # tiles per position that need the j>i mask

_cache = {}


def _build_nc():
    import concourse.mybir as mybir
    from concourse import bacc
    import concourse.tile as tile

    f32 = mybir.dt.float32
    f32r = mybir.dt.float32r
    bf16 = mybir.dt.bfloat16
    Alu = mybir.AluOpType
    ACT = mybir.ActivationFunctionType

    nc = bacc.Bacc("TRN2", target_bir_lowering=False, debug=False,
                   num_devices=NCORES)

    KD = D // P   # 8 chunks of the contraction dim
    NJ = T // P   # 16 key blocks

    # Weights feature-major over the *output* feature e (contraction dim
    # of M = Wq^T Wk and N = Wv^T Wo^T).
    wq_d = nc.dram_tensor("Wqe", [4, P, KD, 256], f32r, kind="ExternalInput")
    wk_d = nc.dram_tensor("Wke", [4, P, KD, 256], f32r, kind="ExternalInput")
    wv_d = nc.dram_tensor("Wve", [4, P, KD, 256], f32r, kind="ExternalInput")
    wo_d = nc.dram_tensor("Wote", [4, P, KD, 256], f32r, kind="ExternalInput")
    # x feature-major f32r stages (global token order) for decay logits,
    # and query-row stages for q~.
    xT_d = nc.dram_tensor("xT", [4, P, KD, 512], f32r, kind="ExternalInput")
    xq_d = nc.dram_tensor("xq", [2, P, KD, 512], f32r, kind="ExternalInput")
    # x pre-cast to bf16: feature-major (keys) and token-major (values).
    xTb_d = nc.dram_tensor("xTb", [P, KD, T], bf16, kind="ExternalInput")
    xtok_d = nc.dram_tensor("xtok", [P, NJ, D], bf16, kind="ExternalInput")
    wd_d = nc.dram_tensor("Wdt", [P, D // P, 2], f32r, kind="ExternalInput")
    negI_d = nc.dram_tensor("negI", [P, NPOS * 256], f32, kind="ExternalInput")
    bd_d = nc.dram_tensor("bd128", [P, 1], f32, kind="ExternalInput")
    os_d = nc.dram_tensor("os128", [P, 1], f32, kind="ExternalInput")
    out_d = nc.dram_tensor("out", [NPOS * 256 // P, 2, P, 512], f32,
                           kind="ExternalOutput")

    blo2 = nc.dram_tensor("blo2", [T], f32)

    with tile.TileContext(nc) as tc:
        with (
            tc.tile_pool(name="resident", bufs=1) as res,
            tc.tile_pool(name="stage", bufs=2) as stage,
            tc.tile_pool(name="spool", bufs=16) as spool,
            tc.tile_pool(name="dwpool", bufs=2) as dwpool,
            tc.tile_pool(name="mpool", bufs=1) as mpool,
            tc.tile_pool(name="rtpool", bufs=1) as rtpool,
            tc.tile_pool(name="opool", bufs=2) as opool,
            tc.tile_pool(name="small", bufs=1) as small,
            tc.tile_pool(name="ldrow", bufs=2) as ldrow_pool,
            tc.tile_pool(name="proj_ps", bufs=2, space="PSUM") as proj_ps,
            tc.tile_pool(name="st_ps", bufs=2, space="PSUM") as st_ps,
            tc.tile_pool(name="ret_ps", bufs=2, space="PSUM") as ret_ps,
            tc.tile_pool(name="out_ps", bufs=2, space="PSUM") as out_ps,
        ):
            qT = res.tile([P, KD, NPOS * 256], bf16)  # 16KB/part
            nT = res.tile([P, KD, D], f32r)           # 32KB/part
            negI = res.tile([P, NPOS * 256], f32)     # 4KB/part

            wd_t = small.tile([P, KD, 2], f32r, tag="wd")
            nc.sync.dma_start(wd_t[:], wd_d.ap())
            bd_t = small.tile([P, 1], f32, tag="bd")
            nc.sync.dma_start(bd_t[:], bd_d.ap())
            os_t = small.tile([P, 1], f32, tag="os")
            nc.sync.dma_start(os_t[:], os_d.ap())
            jall = small.tile([P, NJ], f32, tag="jall")
            nc.gpsimd.iota(jall[:], pattern=[[P, NJ]], base=0,
                           channel_multiplier=1,
                           allow_small_or_imprecise_dtypes=True)
            ldc = small.tile([P, NJ], f32, tag="ldc")
            ldT = small.tile([P, NJ], f32, tag="ldT")
            negLd = small.tile([P, NJ], f32, tag="negld")
            eps_t = small.tile([P, 1], f32, tag="eps")
            nc.vector.memset(eps_t[:], 1e-8)
            os32_t = small.tile([P, 1], f32, tag="os32")
            nc.vector.tensor_scalar_mul(os32_t[:], os_t[:],
                                        1.0 / float(np.sqrt(D)))

            # ---- Phase 1: M = Wq^T Wk, q~ = xq M, decay, N = Wv^T Wo^T --
            with (
                tc.tile_pool(name="wstat", bufs=4) as wstat,
                tc.tile_pool(name="wrhs", bufs=2) as wrhs,
                tc.tile_pool(name="mres", bufs=1) as mres,
            ):
                mT = mres.tile([P, KD, D], f32r)

                def w_quarter(dram, qi):
                    t = wstat.tile([P, KD, 256], f32r, tag="wstat")
                    nc.sync.dma_start(t[:], dram.ap()[qi])
                    return t

                def rhs_quarter(dram, qi):
                    t = wrhs.tile([P, KD, 256], f32r, tag="wrhs")
                    nc.sync.dma_start(t[:], dram.ap()[qi])
                    return t

                def w_quarter_v(dram, qi):
                    # gpsimd-queue DMA: doesn't head-block the sync queue
                    t = wstat.tile([P, KD, 256], f32r, tag="wstat")
                    nc.gpsimd.dma_start(t[:], dram.ap()[qi])
                    return t

                def rhs_quarter_v(dram, qi):
                    t = wrhs.tile([P, KD, 256], f32r, tag="wrhs")
                    nc.gpsimd.dma_start(t[:], dram.ap()[qi])
                    return t

                # M: stationary Wq (all 4 quarters live), rhs Wk streamed.
                wq_q = [w_quarter(wq_d, 0)]
                wk_q0 = rhs_quarter(wk_d, 0)
                wq_q += [w_quarter(wq_d, i) for i in range(1, 4)]
                for cq in range(4):
                    rq = wk_q0 if cq == 0 else rhs_quarter(wk_d, cq)
                    for dch in range(KD):
                        ps = proj_ps.tile([P, 256], f32, tag="proj")
                        wsl = (dch % 2) * P
                        for ech in range(KD):
                            nc.tensor.matmul(
                                ps[:], wq_q[dch // 2][:, ech, wsl:wsl + P],
                                rq[:, ech, :],
                                start=(ech == 0), stop=(ech == KD - 1))
                        nc.vector.tensor_copy(
                            mT[:, dch, cq * 256:(cq + 1) * 256], ps[:])

                # stage N's weights now (vector queue) so the N matmuls
                # aren't starved behind the x stages on the sync queue
                wv_q = [w_quarter_v(wv_d, i) for i in range(4)]
                wo_q01 = [rhs_quarter_v(wo_d, 0), rhs_quarter_v(wo_d, 1)]

                # q~ = xq M (stationary M chunks, rhs xq stages),
                # fold out_scale/sqrt(D) into q~.
                for c in range(2):
                    xs = stage.tile([P, KD, 512], f32r, tag="xs")
                    nc.sync.dma_start(xs[:], xq_d.ap()[c])
                    for fch in range(KD):
                        ps = proj_ps.tile([P, 512], f32, tag="proj")
                        for dch in range(KD):
                            nc.tensor.matmul(
                                ps[:], mT[:, dch, fch * P:(fch + 1) * P],
                                xs[:, dch, :],
                                start=(dch == 0), stop=(dch == KD - 1))
                        nc.scalar.activation(qT[:, fch, c * 512:(c + 1) * 512],
                                             ps[:], ACT.Copy, bias=0.0,
                                             scale=os32_t[:])

                # decay logits over all tokens (global order); row form via
                # DRAM, read back in column form.
                for c in range(4):
                    xs = stage.tile([P, KD, 512], f32r, tag="xs")
                    nc.sync.dma_start(xs[:], xT_d.ap()[c])
                    lp = st_ps.tile([P, 512], f32, tag="st")
                    for od in range(KD):
                        nc.tensor.matmul(lp[0:2, :], wd_t[:, od, :],
                                         xs[:, od, :],
                                         start=(od == 0), stop=(od == KD - 1))
                    lr = ldrow_pool.tile([1, 512], f32, tag="lr")
                    nc.vector.tensor_copy(lr[:], lp[0:1, :])
                    nc.scalar.dma_start(blo2.ap()[c * 512:(c + 1) * 512], lr[:])

                # N: stationary Wv (reuses the wstat ring), rhs Wo^T.
                for gq in range(4):
                    rq = wo_q01[gq] if gq < 2 else rhs_quarter_v(wo_d, gq)
                    for cch in range(KD):
                        ps = proj_ps.tile([P, 256], f32, tag="proj")
                        wsl = (cch % 2) * P
                        for ech in range(KD):
                            nc.tensor.matmul(
                                ps[:], wv_q[cch // 2][:, ech, wsl:wsl + P],
                                rq[:, ech, :],
                                start=(ech == 0), stop=(ech == KD - 1))
                        nc.vector.tensor_copy(
                            nT[:, cch, gq * 256:(gq + 1) * 256], ps[:])

            # decay logits readback (column form) + decay math
            nc.scalar.dma_start(ldc[:],
                                blo2.ap().rearrange("(o p) -> p o", p=P))
            nc.scalar.activation(ldT[:], ldc[:], ACT.Sigmoid,
                                 bias=bd_t[:], scale=1.0)
            nc.scalar.activation(ldT[:], ldT[:], ACT.Ln, bias=eps_t[:])
            nc.vector.tensor_scalar_mul(negLd[:], ldT[:], -1.0)

            nc.sync.dma_start(negI[:], negI_d.ap())

            # ---- Phase 2: attention + output projection per position ----
            with tc.tile_pool(name="xres", bufs=1) as xres:
                xTb = xres.tile([P, KD, T], bf16)     # 32KB/part
                xtok = xres.tile([P, NJ, D], bf16)    # 32KB/part
                nc.sync.dma_start(xTb[:], xTb_d.ap())
                nc.sync.dma_start(xtok[:], xtok_d.ap())

                for k in range(NPOS):
                    isl = slice(k * 256, (k + 1) * 256)
                    rlist = list(range(JSTART[k], NJ))
                    s_tiles = []
                    for t_idx, r in enumerate(rlist):
                        ps = st_ps.tile([P, 256], f32, tag="st")
                        for oe in range(KD):
                            nc.tensor.matmul(
                                ps[:], xTb[:, oe, r * P:(r + 1) * P],
                                qT[:, oe, isl],
                                start=(oe == 0), stop=(oe == KD - 1))
                        # decay weights: dist1 = max(j - i, 0);
                        # w = exp(ld*(dist1-1)); mask = dist1 >= 1
                        dw = dwpool.tile([P, 256], f32, tag="dw")
                        nc.vector.tensor_scalar(dw[:], negI[:, isl],
                                                jall[:, r:r + 1], 0.0,
                                                Alu.add, Alu.max)
                        if t_idx < NMASK:
                            mk = mpool.tile([P, 256], f32, tag="mk")
                            nc.vector.tensor_scalar(mk[:], dw[:], 1.0, None,
                                                    Alu.is_ge)
                        nc.scalar.activation(dw[:], dw[:], ACT.Exp,
                                             bias=negLd[:, r:r + 1],
                                             scale=ldT[:, r:r + 1])
                        if t_idx < NMASK:
                            nc.vector.tensor_mul(dw[:], dw[:], mk[:])
                        s_sb = spool.tile([P, 256], bf16, tag="s")
                        nc.vector.tensor_mul(s_sb[:], ps[:], dw[:])
                        s_tiles.append(s_sb)

                    rt = rtpool.tile([P, KD, 256], f32r, tag="rt")
                    for od in range(KD):
                        rp = ret_ps.tile([P, 256], f32, tag="ret")
                        for t_idx, r in enumerate(rlist):
                            nc.tensor.matmul(
                                rp[:], xtok[:, r, od * P:(od + 1) * P],
                                s_tiles[t_idx][:],
                                start=(t_idx == 0),
                                stop=(t_idx == len(rlist) - 1))
                        nc.vector.tensor_copy(rt[:, od, :], rp[:])

                    for isub in range(2):
                        for ec in range(2):
                            op = out_ps.tile([P, 512], f32, tag="op")
                            for od in range(KD):
                                nc.tensor.matmul(
                                    op[:], rt[:, od, isub * P:(isub + 1) * P],
                                    nT[:, od, ec * 512:(ec + 1) * 512],
                                    start=(od == 0), stop=(od == KD - 1))
                            ob = opool.tile([P, 512], f32, tag="ob")
                            nc.vector.tensor_copy(ob[:], op[:])
                            nc.sync.dma_start(
                                out_d.ap()[2 * k + isub, ec], ob[:])

    nc.compile()
    return nc


def _core_rows(h):
    return np.concatenate(
        [np.arange(256 * m, 256 * (m + 1)) for m in MLIST[h]])


def _dmalayout(arrT, ch=512):
    """[D, ncols] feature-major array -> [ncols//ch, 128, D//128, ch]."""
    d, ncols = arrT.shape
    return np.ascontiguousarray(
        arrT.reshape(d // P, P, ncols // ch, ch).transpose(2, 1, 0, 3))


def make_in_maps(x, Wq, Wk, Wv, Wo, Wd, bd, out_scale):
    import ml_dtypes
    f = np.float32
    bf = ml_dtypes.bfloat16
    x = np.asarray(x, f)
    # contraction of M/N runs over the torch-Linear *output* feature e,
    # i.e. the weights' first axis: ship them untransposed (Wo transposed).
    wqe = _dmalayout(np.asarray(Wq, f), ch=256)
    wke = _dmalayout(np.asarray(Wk, f), ch=256)
    wve = _dmalayout(np.asarray(Wv, f), ch=256)
    wote = _dmalayout(np.asarray(Wo, f).T, ch=256)
    wdt = np.ascontiguousarray(
        np.concatenate([np.asarray(Wd, f).reshape(1, D).T,
                        np.zeros((D, 1), f)], axis=1)
        .reshape(D // P, P, 2).swapaxes(0, 1))
    bd128 = np.full((P, 1), np.asarray(bd, f).reshape(-1)[0], f)
    os128 = np.full((P, 1), np.asarray(out_scale, f).reshape(-1)[0], f)

    in_maps = []
    rows_h = {h: _core_rows(h) for h in (0, 1)}
    negI_h = {h: np.tile(-rows_h[h].astype(f)[None, :], (P, 1))
              for h in (0, 1)}
    for c in range(NCORES):
        b, h = c // 2, c % 2
        xb = x[b]
        xbT = xb.T  # [D, T]
        in_maps.append({
            "xT": _dmalayout(xbT),
            "xq": _dmalayout(np.ascontiguousarray(xb[rows_h[h]].T)),
            "xTb": np.ascontiguousarray(
                xbT.reshape(KD_CONST, P, T).swapaxes(0, 1)).astype(bf),
            "xtok": np.ascontiguousarray(
                xb.reshape(T // P, P, D).swapaxes(0, 1)).astype(bf),
            "Wqe": wqe, "Wke": wke, "Wve": wve, "Wote": wote, "Wdt": wdt,
            "negI": negI_h[h], "bd128": bd128, "os128": os128,
        })
    return in_maps, rows_h


KD_CONST = D // P


def assemble_out(results, rows_h):
    f = np.float32
    out = np.empty((B, T, D), f)
    for c in range(NCORES):
        b, h = c // 2, c % 2
        oc = results[c]["out"]  # [8, 2, 128, 512]
        out[b][rows_h[h]] = oc.transpose(0, 2, 1, 3).reshape(NPOS * 256, D)
    return out


def kernel(x, Wq, Wk, Wv, Wo, Wd, bd, out_scale):
    from concourse.bass_utils import run_bass_kernel_spmd

    if "nc" not in _cache:
        _cache["nc"] = _build_nc()
    nc = _cache["nc"]

    in_maps, rows_h = make_in_maps(x, Wq, Wk, Wv, Wo, Wd, bd, out_scale)
    res = run_bass_kernel_spmd(nc, in_maps, list(range(NCORES)))
    return assemble_out(res.results, rows_h)
